# revision 4
# baseline (speedup 1.0000x reference)
"""Trainium2 Bass kernel for nn_ExpertGather (MoE gather + per-expert GEMM).

Reference computation (B=8, T=8192, I=512, E=16, K=1024, J=512):
    gathered[b,e,k,:] = x[b, Ind[b,e,k], :]
    out[b,e,k,:]      = gathered[b,e,k,:] @ W[e]

Sharding: expert-parallel across 8 NeuronCores. Core c owns experts
[2c, 2c+1]; x is replicated, Ind/W/out are sharded on E.

Per (b, e_local) pair on a core:
  1. SWDGE dma_gather: 1024 token rows (512 f32 = 2 KiB each) from x[b] in
     HBM -> SBUF tile G[128, 8, 512] (token t -> partition t%128, block t//128).
  2. PE transpose (identity matmul) of each [128tok, 128feat] chunk ->
     PSUM [128feat, 128tok]; DVE copy PSUM->SBUF.
  3. 4 accumulating matmuls per token tile (contraction I=512 in 128-chunks),
     lhsT = transposed gather chunk, rhs = W[e] chunk [128, 512], dtype
     float32r (full-rate PE, ~19-bit mantissa) -> PSUM [128tok, 512].
  4. ACT copy PSUM->SBUF, HWDGE store 2 MiB per pair to out[b,e].
"""

import sys

import numpy as np

if "/opt/trn_rl_repo" not in sys.path:
    sys.path.insert(0, "/opt/trn_rl_repo")

B, T, I = 8, 8192, 512
E, K, J = 16, 1024, 512
NCORES = 8
E_LOCAL = E // NCORES  # 2 experts per core
PAIRS = B * E_LOCAL  # 16 (b, e_local) pairs per core
KT = K // 128  # 8 token tiles per pair
IC = I // 128  # 4 contraction chunks
IDX_W = K // 16  # 64 idxs per partition row (16-partition wrap)

# matmul dtype: float32r = fp32 in memory, reduced-precision full-rate PE
# matmul. Set to "float32" for exact-fp32 (4x slower PE) fallback.
MM_DTYPE = "float32r"

_CACHE: dict = {}


def _build_nc():
    import concourse.mybir as mybir
    import concourse.tile as tile
    from concourse import bacc
    from concourse.masks import make_identity

    f32 = mybir.dt.float32
    i16 = mybir.dt.int16
    mm_dt = getattr(mybir.dt, MM_DTYPE)

    nc = bacc.Bacc("TRN2", target_bir_lowering=False, debug=False)
    x = nc.dram_tensor("x", [B, T, I], f32, kind="ExternalInput")
    w = nc.dram_tensor("w", [128, E_LOCAL, IC, J], mm_dt, kind="ExternalInput")
    idx = nc.dram_tensor("idx", [128, PAIRS, IDX_W], i16, kind="ExternalInput")
    out = nc.dram_tensor("out", [B, E_LOCAL, K, J], f32, kind="ExternalOutput")

    with tile.TileContext(nc) as tc:
        with (
            tc.tile_pool(name="const", bufs=1) as const_pool,
            tc.tile_pool(name="g", bufs=3) as g_pool,
            tc.tile_pool(name="gt", bufs=4) as gt_pool,
            tc.tile_pool(name="osb", bufs=2) as o_pool,
            tc.tile_pool(name="gtps", bufs=3, space="PSUM") as gtps_pool,
            tc.tile_pool(name="ops", bufs=2, space="PSUM") as ops_pool,
        ):
            ident = const_pool.tile([128, 128], f32)
            make_identity(nc, ident)
            w_sb = const_pool.tile([128, E_LOCAL, IC, J], mm_dt)
            nc.sync.dma_start(w_sb[:], w[:])
            idx_sb = const_pool.tile([128, PAIRS, IDX_W], i16)
            nc.sync.dma_start(idx_sb[:], idx[:])

            for q in range(PAIRS):
                b, e = divmod(q, E_LOCAL)
                g = g_pool.tile([128, KT, I], f32)
                nc.gpsimd.dma_gather(g[:], x[b], idx_sb[:, q, :], K, K, I)
                o_sb = o_pool.tile([128, KT, J], f32)
                for tt in range(KT):
                    gt_ps = gtps_pool.tile([128, I], f32)
                    for ic in range(IC):
                        nc.tensor.transpose(
                            gt_ps[:, ic * 128 : (ic + 1) * 128],
                            g[:, tt, ic * 128 : (ic + 1) * 128],
                            ident,
                        )
                    gt_sb = gt_pool.tile([128, I], mm_dt)
                    nc.vector.tensor_copy(out=gt_sb[:], in_=gt_ps[:])
                    o_ps = ops_pool.tile([128, J], f32)
                    for ic in range(IC):
                        nc.tensor.matmul(
                            o_ps[:],
                            gt_sb[:, ic * 128 : (ic + 1) * 128],
                            w_sb[:, e, ic, :],
                            start=(ic == 0),
                            stop=(ic == IC - 1),
                        )
                    nc.scalar.copy(out=o_sb[:, tt, :], in_=o_ps[:])
                nc.sync.dma_start(
                    out[b, e].rearrange("(blk p) j -> p blk j", p=128), o_sb[:]
                )
    nc.compile()
    return nc


def _get_nc():
    if "nc" not in _CACHE:
        _CACHE["nc"] = _build_nc()
    return _CACHE["nc"]


def _make_in_maps(x, Ind, W):
    x = np.ascontiguousarray(np.asarray(x, dtype=np.float32))
    Ind = np.asarray(Ind)
    W = np.asarray(W, dtype=np.float32)
    in_maps = []
    for c in range(NCORES):
        wl = W[c * E_LOCAL : (c + 1) * E_LOCAL]  # [E_LOCAL, I, J]
        # w_host[p, e, ic, j] = wl[e, ic*128 + p, j]
        w_host = np.ascontiguousarray(
            wl.reshape(E_LOCAL, IC, 128, J).transpose(2, 0, 1, 3)
        )
        idxs = np.empty((128, PAIRS, IDX_W), np.int16)
        for b in range(B):
            for e in range(E_LOCAL):
                q = b * E_LOCAL + e
                # unwrapped[j] = idxs[j % 16, j // 16]  (16-partition wrap)
                wrapped = Ind[b, c * E_LOCAL + e].astype(np.int16).reshape(IDX_W, 16).T
                idxs[:, q, :] = np.tile(wrapped, (8, 1))
        in_maps.append({"x": x, "w": w_host, "idx": idxs})
    return in_maps


def run(x, Ind, W, trace=False):
    """Run the kernel; returns (out, BassKernelResults)."""
    from concourse.bass_utils import run_bass_kernel_spmd

    nc = _get_nc()
    in_maps = _make_in_maps(x, Ind, W)
    res = run_bass_kernel_spmd(
        nc, in_maps, core_ids=list(range(NCORES)), trace=trace
    )
    outs = [r["out"] for r in res.results]  # each [B, E_LOCAL, K, J]
    full = np.concatenate(outs, axis=1)  # experts in core order -> [B, E, K, J]
    return np.ascontiguousarray(full.astype(np.float32)), res


def kernel(x, Ind, W):
    out, _ = run(x, Ind, W, trace=False)
    return out


# revision 9
# speedup vs baseline: 7.9581x; 7.9581x over previous
"""Trainium2 Bass kernel for nn_ExpertGather (MoE gather + per-expert GEMM).

Reference computation (B=8, T=8192, I=512, E=16, K=1024, J=512):
    gathered[b,e,k,:] = x[b, Ind[b,e,k], :]
    out[b,e,k,:]      = gathered[b,e,k,:] @ W[e]

Sharding: expert-parallel across 8 NeuronCores. Core c owns experts
[2c, 2c+1]; x is replicated, Ind/W/out are sharded on E.

Per (b, e_local) pair on a core:
  1. SWDGE dma_gather: 1024 token rows (512 f32 = 2 KiB each) from x[b] in
     HBM -> SBUF tile G[128, 8, 512] (token t -> partition t%128, block t//128).
  2. PE transpose (identity matmul) of each [128tok, 128feat] chunk ->
     PSUM [128feat, 128tok]; DVE copy PSUM->SBUF.
  3. 4 accumulating matmuls per token tile (contraction I=512 in 128-chunks),
     lhsT = transposed gather chunk, rhs = W[e] chunk [128, 512], dtype
     float32r (full-rate PE, ~19-bit mantissa) -> PSUM [128tok, 512].
  4. ACT copy PSUM->SBUF, HWDGE store 2 MiB per pair to out[b,e].
"""

import sys

import numpy as np

if "/opt/trn_rl_repo" not in sys.path:
    sys.path.insert(0, "/opt/trn_rl_repo")

B, T, I = 8, 8192, 512
E, K, J = 16, 1024, 512
NCORES = 8
E_LOCAL = E // NCORES  # 2 experts per core
PAIRS = B * E_LOCAL  # 16 (b, e_local) pairs per core
KT = K // 128  # 8 token tiles per pair
IC = I // 128  # 4 contraction chunks
IDX_W = K // 16  # 64 idxs per partition row (16-partition wrap)

# matmul dtype: float32r = fp32 in memory, reduced-precision full-rate PE
# matmul. Set to "float32" for exact-fp32 (4x slower PE) fallback.
MM_DTYPE = "float32r"

_CACHE: dict = {}


def _build_nc(repeat=1):
    """Build the Bass module. `repeat` re-emits the whole computation that
    many times inside one NEFF (timing use only: slope between repeat counts
    cancels per-call dispatch overhead)."""
    import concourse.mybir as mybir
    import concourse.tile as tile
    from concourse import bacc
    from concourse.masks import make_identity

    f32 = mybir.dt.float32
    i16 = mybir.dt.int16
    mm_dt = getattr(mybir.dt, MM_DTYPE)

    nc = bacc.Bacc("TRN2", target_bir_lowering=False, debug=False)
    x = nc.dram_tensor("x", [B, T, I], f32, kind="ExternalInput")
    w = nc.dram_tensor("w", [128, E_LOCAL, IC, J], mm_dt, kind="ExternalInput")
    idx = nc.dram_tensor("idx", [128, PAIRS, IDX_W], i16, kind="ExternalInput")
    out = nc.dram_tensor("out", [B, E_LOCAL, K, J], f32, kind="ExternalOutput")

    with tile.TileContext(nc) as tc:
        with (
            tc.tile_pool(name="const", bufs=1) as const_pool,
            tc.tile_pool(name="g", bufs=4) as g_pool,
            tc.tile_pool(name="gt", bufs=6) as gt_pool,
            tc.tile_pool(name="osb", bufs=3) as o_pool,
            tc.tile_pool(name="gtps", bufs=4, space="PSUM") as gtps_pool,
            tc.tile_pool(name="ops", bufs=4, space="PSUM") as ops_pool,
        ):
            ident = const_pool.tile([128, 128], f32)
            make_identity(nc, ident)
            w_sb = const_pool.tile([128, E_LOCAL, IC, J], mm_dt)
            nc.sync.dma_start(w_sb[:], w[:])
            idx_sb = const_pool.tile([128, PAIRS, IDX_W], i16)
            nc.sync.dma_start(idx_sb[:], idx[:])

            for q in range(PAIRS * repeat):
                b, e = divmod(q % PAIRS, E_LOCAL)
                g = g_pool.tile([128, KT, I], f32)
                nc.gpsimd.dma_gather(g[:], x[b], idx_sb[:, q % PAIRS, :], K, K, I)
                o_sb = o_pool.tile([128, KT, J], f32)
                for tt in range(KT):
                    gt_ps = gtps_pool.tile([128, I], f32)
                    for ic in range(IC):
                        nc.tensor.transpose(
                            gt_ps[:, ic * 128 : (ic + 1) * 128],
                            g[:, tt, ic * 128 : (ic + 1) * 128],
                            ident,
                        )
                    gt_sb = gt_pool.tile([128, I], mm_dt)
                    nc.vector.tensor_copy(out=gt_sb[:], in_=gt_ps[:])
                    o_ps = ops_pool.tile([128, J], f32)
                    for ic in range(IC):
                        nc.tensor.matmul(
                            o_ps[:],
                            gt_sb[:, ic * 128 : (ic + 1) * 128],
                            w_sb[:, e, ic, :],
                            start=(ic == 0),
                            stop=(ic == IC - 1),
                        )
                    nc.scalar.copy(out=o_sb[:, tt, :], in_=o_ps[:])
                nc.sync.dma_start(
                    out[b, e].rearrange("(blk p) j -> p blk j", p=128), o_sb[:]
                )
    nc.compile()
    return nc


def _get_nc(repeat=1):
    key = ("nc", repeat)
    if key not in _CACHE:
        _CACHE[key] = _build_nc(repeat)
    return _CACHE[key]


def _make_in_maps(x, Ind, W):
    x = np.ascontiguousarray(np.asarray(x, dtype=np.float32))
    Ind = np.asarray(Ind)
    W = np.asarray(W, dtype=np.float32)
    in_maps = []
    for c in range(NCORES):
        wl = W[c * E_LOCAL : (c + 1) * E_LOCAL]  # [E_LOCAL, I, J]
        # w_host[p, e, ic, j] = wl[e, ic*128 + p, j]
        w_host = np.ascontiguousarray(
            wl.reshape(E_LOCAL, IC, 128, J).transpose(2, 0, 1, 3)
        )
        idxs = np.empty((128, PAIRS, IDX_W), np.int16)
        for b in range(B):
            for e in range(E_LOCAL):
                q = b * E_LOCAL + e
                # unwrapped[j] = idxs[j % 16, j // 16]  (16-partition wrap)
                wrapped = Ind[b, c * E_LOCAL + e].astype(np.int16).reshape(IDX_W, 16).T
                idxs[:, q, :] = np.tile(wrapped, (8, 1))
        in_maps.append({"x": x, "w": w_host, "idx": idxs})
    return in_maps


def run(x, Ind, W, trace=False):
    """Run the kernel; returns (out, BassKernelResults)."""
    from concourse.bass_utils import run_bass_kernel_spmd

    nc = _get_nc()
    in_maps = _make_in_maps(x, Ind, W)
    res = run_bass_kernel_spmd(
        nc, in_maps, core_ids=list(range(NCORES)), trace=trace
    )
    outs = [r["out"] for r in res.results]  # each [B, E_LOCAL, K, J]
    full = np.concatenate(outs, axis=1)  # experts in core order -> [B, E, K, J]
    return np.ascontiguousarray(full.astype(np.float32)), res


def kernel(x, Ind, W):
    out, _ = run(x, Ind, W, trace=False)
    return out


# revision 10
# speedup vs baseline: 9.4277x; 1.1847x over previous
"""Trainium2 Bass kernel for nn_ExpertGather (MoE gather + per-expert GEMM).

Reference computation (B=8, T=8192, I=512, E=16, K=1024, J=512):
    gathered[b,e,k,:] = x[b, Ind[b,e,k], :]
    out[b,e,k,:]      = gathered[b,e,k,:] @ W[e]

Sharding: expert-parallel across 8 NeuronCores. Core c owns experts
[2c, 2c+1]; x is replicated, Ind/W/out are sharded on E.

Per (b, e_local) pair on a core:
  1. SWDGE dma_gather: 1024 token rows (512 f32 = 2 KiB each) from x[b] in
     HBM -> SBUF tile G[128, 8, 512] (token t -> partition t%128, block t//128).
  2. PE transpose (identity matmul) of each [128tok, 128feat] chunk ->
     PSUM [128feat, 128tok]; DVE copy PSUM->SBUF.
  3. 4 accumulating matmuls per token tile (contraction I=512 in 128-chunks),
     lhsT = transposed gather chunk, rhs = W[e] chunk [128, 512], dtype
     float32r (full-rate PE, ~19-bit mantissa) -> PSUM [128tok, 512].
  4. ACT copy PSUM->SBUF, HWDGE store 2 MiB per pair to out[b,e].
"""

import sys

import numpy as np

if "/opt/trn_rl_repo" not in sys.path:
    sys.path.insert(0, "/opt/trn_rl_repo")

B, T, I = 8, 8192, 512
E, K, J = 16, 1024, 512
NCORES = 8
E_LOCAL = E // NCORES  # 2 experts per core
PAIRS = B * E_LOCAL  # 16 (b, e_local) pairs per core
KT = K // 128  # 8 token tiles per pair
IC = I // 128  # 4 contraction chunks
IDX_W = K // 16  # 64 idxs per partition row (16-partition wrap)

# matmul dtype: float32r = fp32 in memory, reduced-precision full-rate PE
# matmul. Set to "float32" for exact-fp32 (4x slower PE) fallback.
MM_DTYPE = "float32r"

_CACHE: dict = {}


def _build_nc(repeat=1):
    """Build the Bass module. `repeat` re-emits the whole computation that
    many times inside one NEFF (timing use only: slope between repeat counts
    cancels per-call dispatch overhead)."""
    import concourse.mybir as mybir
    import concourse.tile as tile
    from concourse import bacc
    from concourse.masks import make_identity

    f32 = mybir.dt.float32
    i16 = mybir.dt.int16
    mm_dt = getattr(mybir.dt, MM_DTYPE)

    nc = bacc.Bacc("TRN2", target_bir_lowering=False, debug=False)
    x = nc.dram_tensor("x", [B, T, I], f32, kind="ExternalInput")
    w = nc.dram_tensor("w", [128, E_LOCAL, IC, J], mm_dt, kind="ExternalInput")
    idx = nc.dram_tensor("idx", [128, PAIRS, IDX_W], i16, kind="ExternalInput")
    out = nc.dram_tensor("out", [B, E_LOCAL, K, J], f32, kind="ExternalOutput")

    with tile.TileContext(nc) as tc:
        with (
            tc.tile_pool(name="const", bufs=1) as const_pool,
            tc.tile_pool(name="g", bufs=4) as g_pool,
            tc.tile_pool(name="gt", bufs=6) as gt_pool,
            tc.tile_pool(name="osb", bufs=3) as o_pool,
            tc.tile_pool(name="gtps", bufs=4, space="PSUM") as gtps_pool,
            tc.tile_pool(name="ops", bufs=4, space="PSUM") as ops_pool,
        ):
            ident = const_pool.tile([128, 128], f32)
            make_identity(nc, ident)
            w_sb = const_pool.tile([128, E_LOCAL, IC, J], mm_dt)
            nc.sync.dma_start(w_sb[:], w[:])
            idx_sb = const_pool.tile([128, PAIRS, IDX_W], i16)
            nc.sync.dma_start(idx_sb[:], idx[:])

            for q in range(PAIRS * repeat):
                b, e = divmod(q % PAIRS, E_LOCAL)
                g = g_pool.tile([128, KT, I], f32)
                nc.gpsimd.dma_gather(g[:], x[b], idx_sb[:, q % PAIRS, :], K, K, I)
                o_sb = o_pool.tile([128, KT, J], f32)
                for tt in range(KT):
                    gt_ps = gtps_pool.tile([128, I], f32)
                    for ic in range(IC):
                        nc.tensor.transpose(
                            gt_ps[:, ic * 128 : (ic + 1) * 128],
                            g[:, tt, ic * 128 : (ic + 1) * 128],
                            ident,
                        )
                    gt_sb = gt_pool.tile([128, I], mm_dt)
                    nc.vector.tensor_copy(out=gt_sb[:], in_=gt_ps[:])
                    o_ps = ops_pool.tile([128, J], f32)
                    for ic in range(IC):
                        nc.tensor.matmul(
                            o_ps[:],
                            gt_sb[:, ic * 128 : (ic + 1) * 128],
                            w_sb[:, e, ic, :],
                            start=(ic == 0),
                            stop=(ic == IC - 1),
                        )
                    nc.scalar.copy(out=o_sb[:, tt, :], in_=o_ps[:])
                nc.sync.dma_start(
                    out[b, e].rearrange("(blk p) j -> p blk j", p=128), o_sb[:]
                )
    nc.compile()
    return nc


def _get_nc(repeat=1):
    key = ("nc", repeat)
    if key not in _CACHE:
        _CACHE[key] = _build_nc(repeat)
    return _CACHE[key]


def _make_in_maps(x, Ind, W):
    x = np.ascontiguousarray(np.asarray(x, dtype=np.float32))
    Ind = np.asarray(Ind)
    W = np.asarray(W, dtype=np.float32)
    in_maps = []
    for c in range(NCORES):
        wl = W[c * E_LOCAL : (c + 1) * E_LOCAL]  # [E_LOCAL, I, J]
        # w_host[p, e, ic, j] = wl[e, ic*128 + p, j]
        w_host = np.ascontiguousarray(
            wl.reshape(E_LOCAL, IC, 128, J).transpose(2, 0, 1, 3)
        )
        idxs = np.empty((128, PAIRS, IDX_W), np.int16)
        for b in range(B):
            for e in range(E_LOCAL):
                q = b * E_LOCAL + e
                # unwrapped[j] = idxs[j % 16, j // 16]  (16-partition wrap)
                wrapped = Ind[b, c * E_LOCAL + e].astype(np.int16).reshape(IDX_W, 16).T
                idxs[:, q, :] = np.tile(wrapped, (8, 1))
        in_maps.append({"x": x, "w": w_host, "idx": idxs})
    return in_maps


def run(x, Ind, W, trace=False):
    """Run the kernel; returns (out, BassKernelResults)."""
    import os

    from concourse.bass_utils import run_bass_kernel_spmd

    nc = _get_nc()
    in_maps = _make_in_maps(x, Ind, W)
    try:
        res = run_bass_kernel_spmd(
            nc, in_maps, core_ids=list(range(NCORES)), trace=trace
        )
    except ModuleNotFoundError:
        # axon NTFF profiling hook absent (no antenv.axon_hooks) — retry
        # with tracing force-disabled.
        os.environ["BASS_NEVER_TRACE"] = "1"
        res = run_bass_kernel_spmd(
            nc, in_maps, core_ids=list(range(NCORES)), trace=False
        )
    outs = [r["out"] for r in res.results]  # each [B, E_LOCAL, K, J]
    full = np.concatenate(outs, axis=1)  # experts in core order -> [B, E, K, J]
    return np.ascontiguousarray(full.astype(np.float32)), res


def kernel(x, Ind, W):
    out, _ = run(x, Ind, W, trace=False)
    return out
